# revision 6
# baseline (speedup 1.0000x reference)
"""Trainium2 Bass kernel for nn_DeformAttn (sparse per-pixel attention).

Computation (per batch b, H=8 heads x 16 ch, S=9 samples, D=16384 pixels):
  qp = Wq@q + bq ; kp = Wk@kv ; vp = Wv@kv + bv
  logits[h,s,d] = sum_{c in head h} qp'[c,d] * kp[c,s,d] * 0.25
  attn = softmax_s(logits); out[c,d] = sum_s attn[h(c),s,d] * vp[c,s,d]
(The bk contribution to logits is constant over s per head, so softmax
removes it — bk is mathematically irrelevant and dropped entirely.)

Sharding: 8 cores = batch(4) x spatial-half(2). No collectives.

v2 design notes (vs the fp32r baseline):
  - q/kv/W converted to fp16 on HOST: halves HBM traffic, makes every
    matmul a bf16-class 1 cyc/row op with FWL weight loads.
  - All four compute engines used. Per-sample elementwise work is routed
    across DVE / ACT / Pool(GpSimd) to balance:
      * t_s = qp*kp_s: KP_DIRECT samples read kp straight from PSUM on
        DVE (1x); the rest are pair-drained by ACT ([C,2,TN] PSUM pair
        tiles -> one fp16 copy) and multiplied on Pool.
      * u_s = vp_s*ae_s: U_DIRECT samples read ae from PSUM on DVE; the
        rest get an ACT drain + Pool multiply.
      * vp: pair-drained by ACT (4 pairs + 1 single).
    Pool and 2-port DVE ops lock each other on the shared SBUF port, so
    DVE work is kept on 1-port (PSUM-operand) ops while Pool runs.
  - ACT drains of PSUM pairs amortize the ~352-cycle fixed ACT cost.
  - softmax over s without max-subtraction (logits ~ +-6): ACT exp,
    sum-over-s + 1/Z broadcast via tiny mask matmuls.
  - PSUM budget (8 banks): pr pool [C,2,TN]x2 = 4, sg pool [C,TN]x2 = 2,
    at chain 1, o accumulator 1.
"""
import os
import sys

for _p in ("/opt/trn_rl_repo", "/root/.axon_site/_ro/trn_rl_repo"):
    if os.path.isdir(_p) and _p not in sys.path:
        sys.path.insert(0, _p)

import numpy as np
from contextlib import ExitStack

import concourse.bass as bass
import concourse.bacc as bacc
import concourse.tile as tile
from concourse import mybir
from concourse.bass_utils import run_bass_kernel_spmd

F32 = mybir.dt.float32
BF16 = mybir.dt.float16  # fp16: 10-bit mantissa, same PE/DVE speed as bf16
AF = mybir.ActivationFunctionType

B, C = 4, 128
H, HC, S = 8, 16, 9
FH, FW = 128, 128
D_FULL = FH * FW          # 16384
D_HALF = D_FULL // 2      # 8192 pixels per core
TN = 512                  # pixels per tile
NT = D_HALF // TN         # 16 tiles
SCALE = HC ** -0.5        # 0.25
N_CORES = 8

# ---- routing config (tunables) ----
KP_DIRECT = (0, 1, 2)          # t-mul reads kp from PSUM on DVE (1x)
KP_PAIRS = ((3, 4), (5, 6), (7, 8))   # ACT pair-drain; t-muls on Pool
KP_POOL = (3, 4, 5, 6, 7, 8)   # of the drained ones, which go to Pool (rest DVE 2x)
U_DIRECT = (0, 1, 2, 3, 4, 5, 6)  # u-mul reads ae from PSUM on DVE (1x)
U_POOL = (7, 8)                # ACT-drained ae pair, u-muls on Pool
VP_PAIRS = ((0, 1), (2, 3), (4, 5), (6, 7))  # ACT pair-drains; vp8 single
AE_PAIRS = ((0, 1), (2, 3), (4, 5), (6, None), (7, 8))  # ae PSUM pair tiles

# bf16 const blob column layout
KM_OFF = 0                 # kmask: S slices of [128, 72]
EM_OFF = KM_OFF + S * 72   # emask: S slices of [72, 128]
GM_OFF = EM_OFF + S * 128  # gmask [72, 8]
B72_OFF = GM_OFF + 8       # b72 [8, 72]
IM_OFF = B72_OFF + 72      # identity [128, 128]
NB = IM_OFF + 128          # bf16 cols

NW = 3 * C                 # fp16 weight blob cols: WqT | WkT | WvT
NF = 2                     # f32 blob: bq | bv


def _build_nc(repeat=1):
    nc = bacc.Bacc("TRN2", target_bir_lowering=False, debug=False,
                   num_devices=N_CORES)
    dp = nc.declare_dram_parameter
    q_d = dp("q", [C, D_HALF], BF16, isOutput=False)
    kv_d = dp("kv", [C, NT, S, TN], BF16, isOutput=False)
    ww_d = dp("blob_w", [C, NW], BF16, isOutput=False)
    bb_d = dp("blob_b", [C, NB], BF16, isOutput=False)
    bf_d = dp("blob_f", [C, NF], F32, isOutput=False)
    out_d = dp("out", [C, D_HALF], F32, isOutput=True)

    with ExitStack() as ctx:
        tc = ctx.enter_context(tile.TileContext(nc))
        p_const = ctx.enter_context(tc.tile_pool(name="consts", bufs=1))
        p_qin = ctx.enter_context(tc.tile_pool(name="qin", bufs=4))
        p_kvin = ctx.enter_context(tc.tile_pool(name="kvin", bufs=4))
        p_qp = ctx.enter_context(tc.tile_pool(name="qp", bufs=3))
        p_kpbf = ctx.enter_context(tc.tile_pool(name="kpbf", bufs=3))
        p_t = ctx.enter_context(tc.tile_pool(name="t", bufs=6))
        p_vp = ctx.enter_context(tc.tile_pool(name="vp", bufs=2))
        p_sm = ctx.enter_context(tc.tile_pool(name="sm", bufs=3))
        p_aebf = ctx.enter_context(tc.tile_pool(name="aebf", bufs=2))
        p_u = ctx.enter_context(tc.tile_pool(name="u", bufs=6))
        p_out = ctx.enter_context(tc.tile_pool(name="outp", bufs=NT))
        # PSUM: pr pool [C,2,TN] x3 bufs = 6 banks; ato (lg|o alternating) = 2
        ps_pr = ctx.enter_context(tc.tile_pool(name="pspr", bufs=3, space="PSUM"))
        ps_ato = ctx.enter_context(tc.tile_pool(name="psato", bufs=2, space="PSUM"))

        # ---- constants (one DMA per blob) ----
        ww_sb = p_const.tile([C, NW], BF16)
        nc.sync.dma_start(ww_sb[:], ww_d[:])
        bb_sb = p_const.tile([C, NB], BF16)
        nc.sync.dma_start(bb_sb[:], bb_d[:])
        bf_sb = p_const.tile([C, NF], F32)
        nc.sync.dma_start(bf_sb[:], bf_d[:])

        # joins: let each engine observe the const DMAs up front so later
        # wait-limited instructions only wait on their streaming operand
        nc.tensor.ldweights(bb_sb[:, 0:128])
        nc.tensor.ldweights(ww_sb[:, 0:128])
        act_join = p_const.tile([C, 1], F32)
        nc.scalar.copy(act_join[:], bf_sb[:, 0:1])
        dve_join = p_const.tile([C, 1], F32)
        nc.vector.tensor_copy(dve_join[:], bf_sb[:, 0:1])
        pool_join = p_const.tile([C, 1], BF16)
        nc.gpsimd.tensor_copy(pool_join[:], bb_sb[:, 0:1])

        wq_t = ww_sb[:, 0:C]
        wk_t = ww_sb[:, C:2 * C]
        wv_t = ww_sb[:, 2 * C:3 * C]
        bq_col = bf_sb[:, 0:1]
        bv_col = bf_sb[:, 1:2]
        gm = bb_sb[0:72, GM_OFF:GM_OFF + 8]
        b72 = bb_sb[0:8, B72_OFF:B72_OFF + 72]
        im = bb_sb[:, IM_OFF:IM_OFF + 128]

        def km(s):
            return bb_sb[:, KM_OFF + s * 72:KM_OFF + (s + 1) * 72]

        def em(s):
            return bb_sb[0:72, EM_OFF + s * 128:EM_OFF + (s + 1) * 128]

        def _body():
          for t in range(NT):
              # ---- loads ----
              q_t = p_qin.tile([C, TN], BF16)
              nc.sync.dma_start(q_t[:], q_d[:, t * TN:(t + 1) * TN])
              kv_t = p_kvin.tile([C, S, TN], BF16)
              nc.sync.dma_start(kv_t[:], kv_d[:, t])

              # ---- q projection + bias; kp0 shares the pair tile ----
              p0 = ps_pr.tile([C, 2, TN], F32, tag="pr")
              nc.tensor.matmul(p0[:, 0], wq_t, q_t[:], start=True, stop=True)
              nc.tensor.matmul(p0[:, 1], wk_t, kv_t[:, 0], start=True, stop=True)
              qp_bf = p_qp.tile([C, TN], BF16, tag="qpb")
              nc.scalar.activation(qp_bf[:], p0[:, 0], AF.Identity, bias=bq_col)

              # kp1/kp2 direct pair
              p1 = ps_pr.tile([C, 2, TN], F32, tag="pr")
              nc.tensor.matmul(p1[:, 0], wk_t, kv_t[:, 1], start=True, stop=True)
              nc.tensor.matmul(p1[:, 1], wk_t, kv_t[:, 2], start=True, stop=True)

              # ---- t = qp*kp for direct samples, logit accumulation ----
              lg_ps = ps_ato.tile([72, TN], F32, tag="ato")
              kp_direct = {0: p0[:, 1], 1: p1[:, 0], 2: p1[:, 1]}
              for s in KP_DIRECT:
                  t_sb = p_t.tile([C, TN], BF16)
                  nc.vector.tensor_mul(t_sb[:], qp_bf[:], kp_direct[s])
                  nc.tensor.matmul(lg_ps[:], km(s), t_sb[:],
                                   start=(s == 0), stop=False)

              # paired kp samples: ACT pair-drain to fp16, muls on Pool
              kp_bfs = {}
              for (sa, sb) in KP_PAIRS:
                  kp_pr = ps_pr.tile([C, 2, TN], F32, tag="pr")
                  nc.tensor.matmul(kp_pr[:, 0], wk_t, kv_t[:, sa], start=True, stop=True)
                  nc.tensor.matmul(kp_pr[:, 1], wk_t, kv_t[:, sb], start=True, stop=True)
                  kp_bf = p_kpbf.tile([C, 2, TN], BF16)
                  nc.scalar.copy(kp_bf[:], kp_pr[:])
                  kp_bfs[sa] = kp_bf[:, 0]
                  kp_bfs[sb] = kp_bf[:, 1]

              # vp pair projections interleave with the drained t-muls:
              # PE stays busy while ACT drains and Pool multiplies
              vp_bf = p_vp.tile([C, S, TN], BF16)
              for pi, (sa, sb) in enumerate(VP_PAIRS):
                  vp_pr = ps_pr.tile([C, 2, TN], F32, tag="pr")
                  nc.tensor.matmul(vp_pr[:, 0], wv_t, kv_t[:, sa], start=True, stop=True)
                  nc.tensor.matmul(vp_pr[:, 1], wv_t, kv_t[:, sb], start=True, stop=True)
                  if pi < len(KP_PAIRS):
                      psa, psb = KP_PAIRS[pi]
                      for s in (psa, psb):
                          t_sb = p_t.tile([C, TN], BF16)
                          if s in KP_POOL:
                              nc.gpsimd.tensor_mul(t_sb[:], qp_bf[:], kp_bfs[s])
                          else:
                              nc.vector.tensor_mul(t_sb[:], qp_bf[:], kp_bfs[s])
                          last = (s == KP_PAIRS[-1][1])
                          nc.tensor.matmul(lg_ps[:], km(s), t_sb[:],
                                           start=False, stop=last)
                  nc.scalar.copy(vp_bf[:, sa:sa + 2], vp_pr[:])

              # ---- softmax over s (no max-subtraction; logits bounded) ----
              exp_sb = p_sm.tile([72, TN], BF16, tag="exp")
              nc.scalar.activation(exp_sb[:], lg_ps[:], AF.Exp, scale=SCALE)

              # vp8 + z share a pair tile; vp8 is PE filler for the
              # softmax latency chain
              p9 = ps_pr.tile([C, 2, TN], F32, tag="pr")
              nc.tensor.matmul(p9[:, 0], wv_t, kv_t[:, 8], start=True, stop=True)
              nc.tensor.matmul(p9[0:8, 1], gm, exp_sb[:], start=True, stop=True)
              nc.scalar.copy(vp_bf[:, 8], p9[:, 0])

              rz_f32 = p_sm.tile([8, TN], F32, tag="rz")
              nc.vector.reciprocal_approx_fast(out=rz_f32[:], in_=p9[0:8, 1])
              rz_sb = p_sm.tile([8, TN], BF16, tag="rzh")
              nc.gpsimd.tensor_copy(rz_sb[:], rz_f32[:])
              p10 = ps_pr.tile([C, 2, TN], F32, tag="pr")
              nc.tensor.matmul(p10[0:72, 0], b72, rz_sb[:], start=True, stop=True)
              # join: absorb the ACT(exp) wait so attn below needs only PE(zb)
              ej_sb = p_sm.tile([8, 1], BF16, tag="ej")
              nc.vector.tensor_copy(ej_sb[:], exp_sb[0:8, 0:1])
              attn_sb = p_sm.tile([72, TN], BF16, tag="attn")
              nc.vector.tensor_mul(attn_sb[:], exp_sb[:], p10[0:72, 0])

              # ---- value phase: expand attn per sample (pair tiles),
              # u = vp*ae, sum over s on PE via identity accumulation ----
              o_ps = ps_ato.tile([C, TN], F32, tag="ato")
              for (sa, sb) in AE_PAIRS:
                  ae_pr = ps_pr.tile([C, 2, TN], F32, tag="pr")
                  nc.tensor.matmul(ae_pr[:, 0], em(sa), attn_sb[:],
                                   start=True, stop=True)
                  if sb is not None:
                      nc.tensor.matmul(ae_pr[:, 1], em(sb), attn_sb[:],
                                       start=True, stop=True)
                  if sa in U_POOL:
                      ae_bf = p_aebf.tile([C, 2, TN], BF16)
                      nc.scalar.copy(ae_bf[:], ae_pr[:])
                  for j, s in enumerate((sa, sb)):
                      if s is None:
                          continue
                      u_sb = p_u.tile([C, TN], BF16)
                      if s in U_POOL:
                          nc.gpsimd.tensor_mul(u_sb[:], vp_bf[:, s], ae_bf[:, j])
                      else:
                          nc.vector.tensor_mul(u_sb[:], vp_bf[:, s], ae_pr[:, j])
                      nc.tensor.matmul(o_ps[:], im, u_sb[:],
                                       start=(s == 0), stop=(s == S - 1))

              out_sb = p_out.tile([C, TN], F32)
              nc.scalar.activation(out_sb[:], o_ps[:], AF.Identity, bias=bv_col)
              nc.sync.dma_start(out_d[:, t * TN:(t + 1) * TN], out_sb[:])
        if repeat == 1:
            _body()
        else:
            with tc.For_i(0, repeat, 1):
                _body()
    nc.compile()
    return nc


def _make_consts(Wq, bq, Wk, bk, Wv, bv):
    bf = np.float16
    blob_w = np.concatenate(
        [np.ascontiguousarray(Wq.T), np.ascontiguousarray(Wk.T),
         np.ascontiguousarray(Wv.T)], axis=1
    ).astype(bf)

    blob_b = np.zeros((C, NB), dtype=bf)
    cc = np.arange(C)
    kmv = np.zeros((C, 72), dtype=np.float32)
    for s in range(S):
        kmv[:] = 0.0
        kmv[cc, (cc // HC) * S + s] = 1.0
        blob_b[:, KM_OFF + s * 72:KM_OFF + (s + 1) * 72] = kmv.astype(bf)
    mm = np.arange(C)
    emv = np.zeros((72, C), dtype=np.float32)
    for s in range(S):
        emv[:] = 0.0
        emv[(mm // HC) * S + s, mm] = 1.0
        blob_b[0:72, EM_OFF + s * 128:EM_OFF + (s + 1) * 128] = emv.astype(bf)
    gmask = np.zeros((72, 8), dtype=np.float32)
    jj = np.arange(72)
    gmask[jj, jj // S] = 1.0
    blob_b[0:72, GM_OFF:GM_OFF + 8] = gmask.astype(bf)
    b72v = np.zeros((8, 72), dtype=np.float32)
    b72v[jj // S, jj] = 1.0
    blob_b[0:8, B72_OFF:B72_OFF + 72] = b72v.astype(bf)
    blob_b[:, IM_OFF:IM_OFF + 128] = np.eye(C, dtype=np.float32).astype(bf)

    blob_f = np.zeros((C, NF), dtype=np.float32)
    blob_f[:, 0] = bq
    blob_f[:, 1] = bv
    return blob_w, blob_b, blob_f


_NC_CACHE = []


def _make_in_maps(q, kv, Wq, bq, Wk, bk, Wv, bv):
    blob_w, blob_b, blob_f = _make_consts(Wq, bq, Wk, bk, Wv, bv)
    q_flat = q.reshape(B, C, D_FULL).astype(np.float16)
    kv_flat = kv.reshape(B, C, S, D_FULL).astype(np.float16)
    in_maps = []
    for core in range(N_CORES):
        b = core // 2
        half = core % 2
        sl = slice(half * D_HALF, (half + 1) * D_HALF)
        q_sh = np.ascontiguousarray(q_flat[b, :, sl])
        kv_sh = np.ascontiguousarray(
            kv_flat[b, :, :, sl].reshape(C, S, NT, TN).transpose(0, 2, 1, 3)
        )                                                  # [C, NT, S, TN]
        in_maps.append({
            "q": q_sh, "kv": kv_sh,
            "blob_w": blob_w, "blob_b": blob_b, "blob_f": blob_f,
        })
    return in_maps


def kernel(q, kv, Wq, bq, Wk, bk, Wv, bv):
    q = np.asarray(q, dtype=np.float32)
    kv = np.asarray(kv, dtype=np.float32)
    args = [np.asarray(a, dtype=np.float32) for a in (Wq, bq, Wk, bk, Wv, bv)]
    in_maps = _make_in_maps(q, kv, *args)

    if not _NC_CACHE:
        _NC_CACHE.append(_build_nc())
    nc = _NC_CACHE[0]
    res = run_bass_kernel_spmd(nc, in_maps, list(range(N_CORES)))

    out = np.empty((B, C, D_FULL), dtype=np.float32)
    for core in range(N_CORES):
        b = core // 2
        half = core % 2
        out[b, :, half * D_HALF:(half + 1) * D_HALF] = res.results[core]["out"]
    return out.reshape(B, C, FH, FW)


if __name__ == "__main__":
    rng = np.random.default_rng(0)
    ins = {
        "q": rng.standard_normal((B, C, FH, FW), dtype=np.float32),
        "kv": rng.standard_normal((B, C, S, D_FULL), dtype=np.float32),
        "Wq": rng.standard_normal((C, C), dtype=np.float32) * C ** -0.5,
        "bq": (rng.standard_normal(C) * 0.01).astype(np.float32),
        "Wk": rng.standard_normal((C, C), dtype=np.float32) * C ** -0.5,
        "bk": (rng.standard_normal(C) * 0.01).astype(np.float32),
        "Wv": rng.standard_normal((C, C), dtype=np.float32) * C ** -0.5,
        "bv": (rng.standard_normal(C) * 0.01).astype(np.float32),
    }
    out = kernel(**ins)
    print("ran, out shape", out.shape, "finite:", np.isfinite(out).all())
